# revision 2
# baseline (speedup 1.0000x reference)
# Cross-attention SDPA kernel for 8 Trainium2 NeuronCores.
#
# reference semantics (per batch b):
#   Q = y @ Wq + bq            [N, 64]
#   K = z @ Wk + bk            [M, 64]
#   V = z @ Wv + bv            [M, 64]
#   O = softmax(Q K^T / 8) V   [N, 64]
# B=4, M=N=4096, D=512.
#
# Sharding: 8 cores = 4 batches x 2 halves of the query (decoder) length.
# Each core sees z^T[b] (full, [512,4096]) and y^T half ([512,2048]),
# pre-transposed and cast to bf16 on the host, and produces O rows
# [2048, 64] fp32.
#
# On-core dataflow (S^T layout so softmax reduction lands on the matmul):
#   zt/yt (d on partitions, 4 chunks of 128)
#   K^T [64,4096]  = sum_c Wk[c].T @ zt[c]   (stored split: m-tiles 0-15 on
#                    partitions 0:64, m-tiles 16-31 on partitions 64:128,
#                    enabling 2-way row-packed S matmuls)
#   Q^T [64,2048]  duplicated to partitions 64:128
#   V   [m,64]     = zt[c]-stationary matmuls, +ones column (denominator)
#   S^T tile pair  = K^T-tile.T @ Q^T (two concurrent row-group matmuls)
#   E = exp(S^T * 0.125)  (ScalarE, PSUM->SBUF bf16)
#   O^T[65, n]    += V_aug.T @ E  (row 64 = softmax denominator)
#   O[n, 64]       = transpose(O^T) * (1/denom) + bv  -> DRAM
#
# bq/bk are folded in as per-partition bias on the PSUM->SBUF copies; bv is
# added at the end (softmax rows sum to 1 so P @ (1 bv^T) == 1 bv^T).

import numpy as np
import ml_dtypes

B = 4
M = 4096
N = 4096
D = 512
KQ = 64
NH = N // 2          # per-core query rows
NCORES = 8
SCALE = 0.125        # 1/sqrt(64)

NT = M // 128        # 32 m-tiles
NPAIR = NT // 2      # 16 row-packed pairs (i, 16+i)
NBLK = 4             # n-blocks of 512 in the attention loop
QBLK = NH // 512     # 4 q-projection blocks
KBLK = M // 512      # 8 k-projection blocks

_CACHE = {}


def build_program():
    from contextlib import ExitStack

    import concourse.bacc as bacc
    import concourse.mybir as mybir
    import concourse.tile as tile
    from concourse.bass import ts, ds
    from concourse.masks import make_identity

    F32 = mybir.dt.float32
    BF16 = mybir.dt.bfloat16
    EXP = mybir.ActivationFunctionType.Exp

    nc = bacc.Bacc("TRN2", target_bir_lowering=False, debug=False)

    zt_d = nc.dram_tensor("zt", [4, 128, M], BF16, kind="ExternalInput").ap()
    yt_d = nc.dram_tensor("yt", [4, 128, NH], BF16, kind="ExternalInput").ap()
    wq_d = nc.dram_tensor("wq", [4, 128, KQ], BF16, kind="ExternalInput").ap()
    wk_d = nc.dram_tensor("wk", [4, 128, KQ], BF16, kind="ExternalInput").ap()
    wv_d = nc.dram_tensor("wv", [4, 128, KQ], BF16, kind="ExternalInput").ap()
    bq_d = nc.dram_tensor("bq", [KQ, 1], F32, kind="ExternalInput").ap()
    bk_d = nc.dram_tensor("bk", [KQ, 1], F32, kind="ExternalInput").ap()
    bv_d = nc.dram_tensor("bv", [1, KQ], F32, kind="ExternalInput").ap()
    o_d = nc.dram_tensor("o", [NH, KQ], F32, kind="ExternalOutput").ap()

    with ExitStack() as ctx:
        tc = ctx.enter_context(tile.TileContext(nc))
        singles = ctx.enter_context(tc.tile_pool(name="singles", bufs=1))
        epool = ctx.enter_context(tc.tile_pool(name="epool", bufs=3))
        otpool = ctx.enter_context(tc.tile_pool(name="otpool", bufs=2))
        ostage = ctx.enter_context(tc.tile_pool(name="ostage", bufs=3))
        rpool = ctx.enter_context(tc.tile_pool(name="rpool", bufs=3))
        spool = ctx.enter_context(tc.tile_pool(name="spool", bufs=2, space="PSUM"))
        opool = ctx.enter_context(tc.tile_pool(name="opool", bufs=1, space="PSUM"))
        ppool = ctx.enter_context(tc.tile_pool(name="ppool", bufs=2, space="PSUM"))

        # --- constants / small inputs ---
        ident = singles.tile([128, 128], F32, name="ident", tag="ident")
        make_identity(nc, ident)
        bq_sb = singles.tile([KQ, 1], F32, name="bq", tag="bq")
        nc.sync.dma_start(bq_sb, bq_d)
        bk_sb = singles.tile([128, 1], F32, name="bk", tag="bk")
        nc.sync.dma_start(bk_sb[0:64, :], bk_d)
        nc.sync.dma_start(bk_sb[64:128, :], bk_d)
        bv_sb = singles.tile([128, KQ], F32, name="bv", tag="bv")
        nc.sync.dma_start(bv_sb, bv_d.to_broadcast((128, KQ)))

        wq_sb = []
        wk_sb = []
        wv_sb = []
        for c in range(4):
            wq_sb.append(singles.tile([128, KQ], BF16, name=f"wq{c}", tag=f"wq{c}"))
            nc.sync.dma_start(wq_sb[c], wq_d[c])
            wk_sb.append(singles.tile([128, KQ], BF16, name=f"wk{c}", tag=f"wk{c}"))
            nc.sync.dma_start(wk_sb[c], wk_d[c])
            wv_sb.append(singles.tile([128, KQ], BF16, name=f"wv{c}", tag=f"wv{c}"))
            nc.sync.dma_start(wv_sb[c], wv_d[c])

        # --- activations in, d on partitions ---
        yt = []
        for c in range(4):
            yt.append(singles.tile([128, NH], BF16, name=f"yt{c}", tag=f"yt{c}"))
            nc.sync.dma_start(yt[c], yt_d[c])
        zt = [[None, None] for _ in range(4)]
        for c in range(4):
            for h in range(2):
                t = singles.tile([128, 2048], BF16, name=f"zt{c}{h}", tag=f"zt{c}{h}")
                zt[c][h] = t
                nc.sync.dma_start(t, zt_d[c, :, ds(h * 2048, 2048)])

        # --- Q^T projection: QT_blk[j] [128, 512], rows duplicated ---
        qt_blk = [singles.tile([128, 512], BF16, name=f"qt{j}", tag=f"qt{j}") for j in range(QBLK)]
        for j in range(QBLK):
            q_ps = ppool.tile([128, 512], F32, name="proj", tag="proj")
            for c in range(4):
                nc.tensor.matmul(
                    q_ps[0:64, :],
                    lhsT=wq_sb[c],
                    rhs=yt[c][:, ts(j, 512)],
                    start=(c == 0),
                    stop=(c == 3),
                )
            nc.vector.tensor_scalar_add(qt_blk[j][0:64, :], q_ps[0:64, :], bq_sb)
            nc.sync.dma_start(qt_blk[j][64:128, :], qt_blk[j][0:64, :])

        # --- K^T projection: KT_blk[j] [128, 512]; lo half m 0..2047 on
        # partitions 0:64, hi half m 2048..4095 on partitions 64:128 ---
        kt_blk = [singles.tile([128, 512], BF16, name=f"kt{j}", tag=f"kt{j}") for j in range(4)]
        for j in range(KBLK):
            h, jj = divmod(j, 4)
            k_ps = ppool.tile([128, 512], F32, name="proj", tag="proj")
            half = slice(0, 64) if h == 0 else slice(64, 128)
            tp = (0, 0) if h == 0 else (0, 64)
            for c in range(4):
                nc.tensor.matmul(
                    k_ps[half, :],
                    lhsT=wk_sb[c],
                    rhs=zt[c][h][:, ts(jj, 512)],
                    start=(c == 0),
                    stop=(c == 3),
                    tile_position=tp,
                )
            nc.vector.tensor_scalar_add(
                kt_blk[jj][half, :], k_ps[half, :], bk_sb[half, :]
            )

        # --- V projection (natural layout) + ones column ---
        v_sb = []
        for t in range(NT):
            h, tt = divmod(t, 16)
            v_ps = ppool.tile([128, 512], F32, name="proj", tag="proj")
            for c in range(4):
                nc.tensor.matmul(
                    v_ps[:, 0:KQ],
                    lhsT=zt[c][h][:, ts(tt, 128)],
                    rhs=wv_sb[c],
                    start=(c == 0),
                    stop=(c == 3),
                )
            vt = singles.tile([128, KQ + 1], BF16, name=f"v{t}", tag=f"v{t}")
            v_sb.append(vt)
            nc.vector.tensor_copy(vt[:, 0:KQ], v_ps[:, 0:KQ])
            nc.vector.memset(vt[:, KQ : KQ + 1], 1.0)

        # --- attention loop ---
        for nb in range(NBLK):
            o_ps = opool.tile([128, 512], F32, name="o", tag="o")
            qlo = qt_blk[nb][0:64, :]
            qhi = qt_blk[nb][64:128, :]
            for i in range(NPAIR):
                jj, col = divmod(i, 4)
                s_ps = spool.tile([128, 1024], F32, name="s", tag="s")
                nc.tensor.matmul(
                    s_ps[:, 0:512],
                    lhsT=kt_blk[jj][0:64, ts(col, 128)],
                    rhs=qlo,
                    start=True,
                    stop=True,
                    tile_position=(0, 0),
                )
                nc.tensor.matmul(
                    s_ps[:, 512:1024],
                    lhsT=kt_blk[jj][64:128, ts(col, 128)],
                    rhs=qhi,
                    start=True,
                    stop=True,
                    tile_position=(64, 0),
                )
                e_t = epool.tile([128, 1024], BF16, name="e", tag="e")
                nc.scalar.activation(e_t, s_ps, EXP, scale=SCALE)
                nc.tensor.matmul(
                    o_ps[0:65, :],
                    lhsT=v_sb[i],
                    rhs=e_t[:, 0:512],
                    start=(i == 0),
                    stop=False,
                )
                nc.tensor.matmul(
                    o_ps[0:65, :],
                    lhsT=v_sb[16 + i],
                    rhs=e_t[:, 512:1024],
                    start=False,
                    stop=(i == NPAIR - 1),
                )

            # --- finalize this n-block: transpose, normalize, bias, store ---
            ot_sb = otpool.tile([128, 512], F32, name="ot", tag="ot")
            nc.vector.tensor_copy(ot_sb[0:65, :], o_ps[0:65, :])
            for s in range(4):
                ot_ps = ppool.tile([128, 512], F32, name="proj", tag="proj")
                nc.tensor.matmul(
                    ot_ps[:, 0:65],
                    lhsT=ot_sb[0:65, ts(s, 128)],
                    rhs=ident[0:65, 0:65],
                    is_transpose=True,
                    start=True,
                    stop=True,
                )
                rcp = rpool.tile([128, 1], F32, name="rcp", tag="rcp")
                nc.vector.reciprocal(rcp, ot_ps[:, 64:65])
                o_st = ostage.tile([128, KQ], F32, name="ost", tag="ost")
                nc.vector.tensor_scalar_mul(o_st, ot_ps[:, 0:KQ], rcp)
                nc.vector.tensor_add(o_st, o_st, bv_sb)
                nc.sync.dma_start(o_d[ds(nb * 512 + s * 128, 128), :], o_st)

    nc.compile()
    return nc


def _get_program():
    if "nc" not in _CACHE:
        _CACHE["nc"] = build_program()
    return _CACHE["nc"]


def make_in_maps(z, y, Wq, bq, Wk, bk, Wv, bv):
    bf16 = ml_dtypes.bfloat16
    zt = np.ascontiguousarray(z.astype(bf16).transpose(0, 2, 1))  # [B, 512, M]
    yt = np.ascontiguousarray(y.astype(bf16).transpose(0, 2, 1))  # [B, 512, N]
    wq = np.ascontiguousarray(Wq.astype(bf16).reshape(4, 128, KQ))
    wk = np.ascontiguousarray(Wk.astype(bf16).reshape(4, 128, KQ))
    wv = np.ascontiguousarray(Wv.astype(bf16).reshape(4, 128, KQ))
    bq2 = np.ascontiguousarray(bq.astype(np.float32).reshape(KQ, 1))
    bk2 = np.ascontiguousarray(bk.astype(np.float32).reshape(KQ, 1))
    bv2 = np.ascontiguousarray(bv.astype(np.float32).reshape(1, KQ))
    in_maps = []
    for c in range(NCORES):
        b, h = divmod(c, 2)
        in_maps.append(
            {
                "zt": zt[b].reshape(4, 128, M),
                "yt": np.ascontiguousarray(
                    yt[b][:, h * NH : (h + 1) * NH]
                ).reshape(4, 128, NH),
                "wq": wq,
                "wk": wk,
                "wv": wv,
                "bq": bq2,
                "bk": bk2,
                "bv": bv2,
            }
        )
    return in_maps


def kernel(z, y, Wq, bq, Wk, bk, Wv, bv):
    from concourse import bass_utils

    nc = _get_program()
    in_maps = make_in_maps(z, y, Wq, bq, Wk, bk, Wv, bv)
    res = bass_utils.run_bass_kernel_spmd(nc, in_maps, core_ids=list(range(NCORES)))
    out = np.empty((B, N, KQ), dtype=np.float32)
    for c in range(NCORES):
        b, h = divmod(c, 2)
        out[b, h * NH : (h + 1) * NH, :] = res.results[c]["o"]
    return out
